# revision 11
# baseline (speedup 1.0000x reference)
"""Trainium2 Bass kernel for Gaussian KDE evaluation.

reference math:
    val[m] = (1/N) * sum_n exp(t1 - 0.5*d2(m,n)/bw^2)
    d2(m,n) = |e_m|^2 + |b_n|^2 - 2<e_m, b_n>
    t1 = -0.5*D*log(2*pi) - log_bw,  bw^2 = exp(2*log_bw)

Strategy (8 NeuronCores, x_eval row-sharded, x_base/log_bw replicated):
  The PE computes |b|^2 - 2<e,b> with bf16 operands split hi/lo for
  fp32-grade accuracy (compensated product: e.b ~ e_hi.b_hi + e_lo.b_hi
  + e_hi.b_lo, |b|^2 = sqb_hi + sqb_lo).  The terms are packed into one
  K=64 matmul per 128x512 output block:

    lhsT rows: [ hi(-2e^T) | lo(-2e^T) | hi(-2e^T) | 1 | 1 | 0-pad ]
    rhs  rows: [ hi(b^T)   | hi(b^T)   | lo(b^T)   | sqb_hi | sqb_lo | 0 ]

  Matmul cost is per *moving column*, so K=64 costs the same as K=17
  fp32 but streams at bf16 rate (1 PE cycle/col, 4x faster than fp32).

  The 64-row sections are built in natural [point, col] bf16 layout
  (free-dim writes have no partition-alignment constraint; compute
  engine APs must start at partition multiples of 32), then one bf16 PE
  transpose flips TWO 64-col tile groups at once into PSUM, and a
  bit-exact DVE copy (bf16 data viewed as f32 words) moves them to
  SBUF.  Per (row-tile, 2048-col chunk) a single ScalarE ACTIVATE
  computes exp(scale*psum + bias) over 4 PSUM banks (scale =
  -0.5/bw^2, bias = t1 - ln(N) + scale*|e|^2) and its accum_out emits
  the row-sum for free.  Two 4-bank PSUM tiles ping-pong between PE
  fill and ScalarE exp+accumulate; base prep is sliced per chunk so the
  ScalarE starts within a few us of kernel start.
"""

import numpy as np

M, N, D = 8192, 16384, 16
NCORES = 8
MS = M // NCORES          # eval rows per core
RT = MS // 128            # row tiles per core (128 evals each)
CH = 2048                 # base points per chunk (4 PSUM banks)
NCH = N // CH             # chunks per core
NBT = N // 128            # 128-row base tiles per core
SLABS = CH // 256         # PE transposes per chunk (2 tiles each)
TPC = NBT // NCH          # 128-pt tiles per chunk (16)
LOG_2PI = float(np.log(2.0 * np.pi))

_CACHE = {}


def _build_nc():
    from concourse import bacc, mybir, masks, tile

    f32 = mybir.dt.float32
    bf16 = mybir.dt.bfloat16
    nc = bacc.Bacc("TRN2", target_bir_lowering=False, debug=False,
                   num_devices=NCORES)

    x_eval = nc.dram_tensor("x_eval", [MS, D], f32, kind="ExternalInput")
    x_base = nc.dram_tensor("x_base", [N, D], f32, kind="ExternalInput")
    log_bw = nc.dram_tensor("log_bw", [1, 1], f32, kind="ExternalInput")
    out = nc.dram_tensor("out", [128, RT], f32, kind="ExternalOutput")

    Exp = mybir.ActivationFunctionType.Exp
    ADD = mybir.AluOpType.add
    SUB = mybir.AluOpType.subtract
    MULT = mybir.AluOpType.mult
    X = mybir.AxisListType.X
    # constant bias: t1 - ln(N); -log_bw and scale*|e|^2 added on-device
    c0 = -0.5 * D * LOG_2PI - float(np.log(N))

    with tile.TileContext(nc) as tc:
        with (
            tc.tile_pool(name="persist", bufs=1) as pp,
            tc.tile_pool(name="rhs", bufs=2) as rhsp,
            tc.tile_pool(name="mm", bufs=2, space="PSUM") as mmp,
        ):
            identity = pp.tile([128, 128], f32)
            masks.make_identity(nc, identity[:])
            idb = pp.tile([128, 128], bf16)
            nc.vector.tensor_copy(idb[:], identity[:])

            setup_ps = mmp.tile([128, CH], f32, tag="mm")

            # ---- log_bw -> per-partition scale/bias columns -------------
            ones_row = pp.tile([1, 128], f32)
            nc.vector.memset(ones_row[:], 1.0)
            lb_sb = pp.tile([1, 1], f32)
            nc.sync.dma_start(out=lb_sb[:], in_=log_bw[:])
            nc.tensor.matmul(setup_ps[:, 1536:1537], ones_row[:], lb_sb[:],
                             start=True, stop=True)
            # scale = -0.5 * exp(-2*log_bw)
            inv_bw2 = pp.tile([128, 1], f32)
            nc.scalar.activation(inv_bw2[:], setup_ps[:, 1536:1537], Exp,
                                 scale=-2.0)
            scale_col = pp.tile([128, 1], f32)
            nc.vector.tensor_scalar_mul(scale_col[:], inv_bw2[:], -0.5)
            # c_col = c0 - log_bw
            c_col = pp.tile([128, 1], f32)
            nc.vector.tensor_scalar(out=c_col[:], in0=setup_ps[:, 1536:1537],
                                    scalar1=-1.0, scalar2=c0,
                                    op0=MULT, op1=ADD)

            # ---- eval-side setup ----------------------------------------
            ev_nat = pp.tile([128, RT * D], f32)
            nc.sync.dma_start(
                out=ev_nat[:].rearrange("p (t d) -> p t d", d=D),
                in_=x_eval[:].rearrange("(p t) d -> p t d", p=128))
            ev_sq = pp.tile([128, RT * D], f32)
            nc.vector.tensor_mul(ev_sq[:], ev_nat[:], ev_nat[:])
            sq_e = pp.tile([128, RT], f32)
            nc.vector.tensor_reduce(
                out=sq_e[:], in_=ev_sq[:].rearrange("p (t d) -> p t d", d=D),
                axis=X, op=ADD)
            # bias_all[:, rt] = scale*|e|^2 + (c0 - log_bw)
            bias_all = pp.tile([128, RT], f32)
            nc.vector.tensor_scalar(out=bias_all[:], in0=sq_e[:],
                                    scalar1=scale_col[:, 0:1],
                                    scalar2=c_col[:, 0:1],
                                    op0=MULT, op1=ADD)

            # ev_ext[:, rt, :] (bf16): cols 0-15 hi(-2e), 16-31 lo(-2e),
            # 32-47 hi(-2e), 48 = 49 = 1.0, 50-63 zero pad
            m2e = pp.tile([128, RT * D], f32)
            nc.vector.tensor_scalar_mul(m2e[:], ev_nat[:], -2.0)
            ev_ext = pp.tile([128, RT * 64], bf16)
            evv = ev_ext[:].rearrange("p (t s) -> p t s", s=64)
            m2e3 = m2e[:].rearrange("p (t d) -> p t d", d=D)
            nc.vector.memset(evv[:, :, 48:64], 0.0)
            nc.vector.memset(evv[:, :, 48:50], 1.0)
            nc.vector.tensor_copy(evv[:, :, 0:16], m2e3)
            nc.vector.tensor_tensor(out=evv[:, :, 16:32], in0=m2e3,
                                    in1=evv[:, :, 0:16], op=SUB)
            nc.vector.tensor_copy(evv[:, :, 32:48], evv[:, :, 0:16])
            for rt in range(RT):
                nc.tensor.transpose(
                    setup_ps[0:64, rt * 64:(rt + 1) * 64].bitcast(bf16),
                    ev_ext[:, rt * 64:(rt + 1) * 64], idb[:])
            # rows 64-127 duplicate rows 0-63: matmul operands must share a
            # base partition, so the B-half matmuls read evT[64:128]
            evT = pp.tile([128, MS], bf16)
            nc.vector.tensor_copy(evT[0:64, :].bitcast(f32),
                                  setup_ps[0:64, 0:RT * 64])
            nc.vector.tensor_copy(evT[64:128, :], evT[0:64, :])

            # ---- base-side prep (per-chunk slices, bf16) ----------------
            # bs_ext[:, t, :]: cols 0-15 hi(b), 16-31 hi(b), 32-47 lo(b),
            # 48 sqb_hi, 49 sqb_lo, 50-63 zero pad
            bs_nat = pp.tile([128, NBT * D], f32)
            nc.sync.dma_start(
                out=bs_nat[:].rearrange("p (t d) -> p t d", d=D),
                in_=x_base[:].rearrange("(p t) d -> p t d", p=128))
            bs_sq = pp.tile([128, NBT * D], f32)
            bs_ext = pp.tile([128, NBT * 64], bf16)
            sq_b = pp.tile([128, NBT], f32)

            def prep_base(ci):
                ts = slice(ci * TPC, (ci + 1) * TPC)
                bsv = bs_ext[:].rearrange("p (t s) -> p t s", s=64)[:, ts]
                nat = bs_nat[:].rearrange("p (t d) -> p t d", d=D)[:, ts]
                sq3 = bs_sq[:].rearrange("p (t d) -> p t d", d=D)[:, ts]
                sqb = sq_b[:].rearrange("p (t o) -> p t o", o=1)[:, ts]
                nc.vector.memset(bsv[:, :, 48:64], 0.0)
                nc.vector.tensor_copy(bsv[:, :, 0:16], nat)
                nc.vector.tensor_copy(bsv[:, :, 16:32], bsv[:, :, 0:16])
                nc.vector.tensor_tensor(out=bsv[:, :, 32:48], in0=nat,
                                        in1=bsv[:, :, 0:16], op=SUB)
                nc.vector.tensor_mul(sq3, nat, nat)
                nc.vector.tensor_reduce(out=sqb, in_=sq3, axis=X, op=ADD)
                nc.vector.tensor_copy(bsv[:, :, 48:49], sqb)
                nc.vector.tensor_tensor(out=bsv[:, :, 49:50], in0=sqb,
                                        in1=bsv[:, :, 48:49], op=SUB)

            prep_base(0)

            # ---- main loop ----------------------------------------------
            sums = pp.tile([128, RT * NCH], f32)
            for ci in range(NCH):
                tA = mmp.tile([128, CH], f32, tag="mm")
                tB = mmp.tile([128, CH], f32, tag="mm")
                rhs = rhsp.tile([128, CH // 2], bf16, tag="rhs")
                # stage two 64-section tile groups per bf16 transpose into
                # the first half of tA (bf16 view), then move bit-exact
                for s in range(SLABS):
                    nc.tensor.transpose(
                        tA[:, s * 64:(s + 1) * 64].bitcast(bf16),
                        bs_ext[:, (ci * SLABS + s) * 128:
                               (ci * SLABS + s + 1) * 128],
                        idb[:])
                nc.vector.tensor_copy(rhs[:].bitcast(f32), tA[:, 0:CH // 4])
                for rt in range(RT):
                    ps = tA if rt % 2 == 0 else tB
                    lhsA = evT[0:64, rt * 128:(rt + 1) * 128]
                    lhsB = evT[64:128, rt * 128:(rt + 1) * 128]
                    half = CH // 4      # 512
                    nc.tensor.matmul(ps[:, 0:half], lhsA,
                                     rhs[0:64, 0:half],
                                     start=True, stop=True)
                    nc.tensor.matmul(ps[:, half:2 * half], lhsA,
                                     rhs[0:64, half:2 * half],
                                     start=True, stop=True)
                    nc.tensor.matmul(ps[:, 2 * half:3 * half], lhsB,
                                     rhs[64:128, 0:half],
                                     start=True, stop=True)
                    nc.tensor.matmul(ps[:, 3 * half:4 * half], lhsB,
                                     rhs[64:128, half:2 * half],
                                     start=True, stop=True)
                    nc.scalar.activation(
                        ps[:, :], ps[:, :], Exp,
                        bias=bias_all[:, rt:rt + 1],
                        scale=scale_col[:, 0:1],
                        accum_out=sums[:, rt * NCH + ci:rt * NCH + ci + 1])
                    if rt == 0 and ci + 1 < NCH:
                        prep_base(ci + 1)

            # ---- finalize -----------------------------------------------
            val = pp.tile([128, RT], f32)
            nc.vector.tensor_reduce(
                out=val[:],
                in_=sums[:].rearrange("p (r c) -> p r c", c=NCH),
                axis=X, op=ADD)
            nc.sync.dma_start(out=out[:], in_=val[:])

    nc.compile()
    return nc


def kernel(x_eval, x_base, log_bw):
    from concourse.bass_utils import run_bass_kernel_spmd

    if "nc" not in _CACHE:
        _CACHE["nc"] = _build_nc()
    nc = _CACHE["nc"]

    x_eval = np.ascontiguousarray(x_eval, dtype=np.float32)
    x_base = np.ascontiguousarray(x_base, dtype=np.float32)
    lb = np.asarray(log_bw, dtype=np.float32).reshape(1, 1)
    in_maps = [
        {
            "x_eval": x_eval[i * MS:(i + 1) * MS],
            "x_base": x_base,
            "log_bw": lb,
        }
        for i in range(NCORES)
    ]
    res = run_bass_kernel_spmd(nc, in_maps, list(range(NCORES)))
    # out[p, rt] holds eval point p*RT + rt of the shard -> row-major flatten
    shards = [r["out"].reshape(-1) for r in res.results]
    return np.concatenate(shards).astype(np.float32)


# revision 14
# speedup vs baseline: 1.1798x; 1.1798x over previous
"""Trainium2 Bass kernel for Gaussian KDE evaluation.

reference math:
    val[m] = (1/N) * sum_n exp(t1 - 0.5*d2(m,n)/bw^2)
    d2(m,n) = |e_m|^2 + |b_n|^2 - 2<e_m, b_n>
    t1 = -0.5*D*log(2*pi) - log_bw,  bw^2 = exp(2*log_bw)

Strategy (8 NeuronCores, x_eval row-sharded, x_base/log_bw replicated):
  The PE computes |b|^2 - 2<e,b> with bf16 operands split hi/lo for
  fp32-grade accuracy (compensated product: e.b ~ e_hi.b_hi + e_lo.b_hi
  + e_hi.b_lo, |b|^2 = sqb_hi + sqb_lo).  The terms are packed into one
  K=64 matmul per 128x512 output block:

    lhsT rows: [ hi(-2e^T) | lo(-2e^T) | hi(-2e^T) | 1 | 1 | 0-pad ]
    rhs  rows: [ hi(b^T)   | hi(b^T)   | lo(b^T)   | sqb_hi | sqb_lo | 0 ]

  Matmul cost is per *moving column*, so K=64 costs the same as K=17
  fp32 but streams at bf16 rate (1 PE cycle/col, 4x faster than fp32).

  The 64-row sections are built in natural [point, col] bf16 layout
  (free-dim writes have no partition-alignment constraint; compute
  engine APs must start at partition multiples of 32), then one bf16 PE
  transpose flips TWO 64-col tile groups at once into PSUM, and a
  bit-exact DVE copy (bf16 data viewed as f32 words) moves them to
  SBUF.  Per (row-tile, 2048-col chunk) a single ScalarE ACTIVATE
  computes exp(scale*psum + bias) over 4 PSUM banks (scale =
  -0.5/bw^2, bias = t1 - ln(N) + scale*|e|^2) and its accum_out emits
  the row-sum for free.  Two 4-bank PSUM tiles ping-pong between PE
  fill and ScalarE exp+accumulate; base prep is sliced per chunk so the
  ScalarE starts within a few us of kernel start.
"""

import numpy as np

M, N, D = 8192, 16384, 16
NCORES = 8
MS = M // NCORES          # eval rows per core
RT = MS // 128            # row tiles per core (128 evals each)
CH = 2048                 # base points per chunk (4 PSUM banks)
NCH = N // CH             # chunks per core
NBT = N // 128            # 128-row base tiles per core
SLABS = CH // 256         # PE transposes per chunk (2 tiles each)
TPC = NBT // NCH          # 128-pt tiles per chunk (16)
LOG_2PI = float(np.log(2.0 * np.pi))

_CACHE = {}


def _build_nc():
    from concourse import bacc, mybir, masks, tile

    f32 = mybir.dt.float32
    bf16 = mybir.dt.bfloat16
    nc = bacc.Bacc("TRN2", target_bir_lowering=False, debug=False,
                   num_devices=NCORES)

    x_eval = nc.dram_tensor("x_eval", [MS, D], f32, kind="ExternalInput")
    x_base = nc.dram_tensor("x_base", [N, D], f32, kind="ExternalInput")
    log_bw = nc.dram_tensor("log_bw", [1, 1], f32, kind="ExternalInput")
    out = nc.dram_tensor("out", [128, RT], f32, kind="ExternalOutput")

    Exp = mybir.ActivationFunctionType.Exp
    ADD = mybir.AluOpType.add
    SUB = mybir.AluOpType.subtract
    MULT = mybir.AluOpType.mult
    X = mybir.AxisListType.X
    # constant bias: t1 - ln(N); -log_bw and scale*|e|^2 added on-device
    c0 = -0.5 * D * LOG_2PI - float(np.log(N))

    with tile.TileContext(nc) as tc:
        with (
            tc.tile_pool(name="persist", bufs=1) as pp,
            tc.tile_pool(name="rhs", bufs=2) as rhsp,
            tc.tile_pool(name="mm", bufs=2, space="PSUM") as mmp,
        ):
            identity = pp.tile([128, 128], f32)
            masks.make_identity(nc, identity[:])
            idb = pp.tile([128, 128], bf16)
            nc.vector.tensor_copy(idb[:], identity[:])

            setup_ps = mmp.tile([128, CH], f32, tag="mm")

            # ---- log_bw -> per-partition scale/bias columns -------------
            ones_row = pp.tile([1, 128], f32)
            nc.vector.memset(ones_row[:], 1.0)
            lb_sb = pp.tile([1, 1], f32)
            nc.sync.dma_start(out=lb_sb[:], in_=log_bw[:])
            nc.tensor.matmul(setup_ps[:, 1536:1537], ones_row[:], lb_sb[:],
                             start=True, stop=True)
            # scale = -0.5 * exp(-2*log_bw)
            inv_bw2 = pp.tile([128, 1], f32)
            nc.scalar.activation(inv_bw2[:], setup_ps[:, 1536:1537], Exp,
                                 scale=-2.0)
            scale_col = pp.tile([128, 1], f32)
            nc.vector.tensor_scalar_mul(scale_col[:], inv_bw2[:], -0.5)
            # c_col = c0 - log_bw
            c_col = pp.tile([128, 1], f32)
            nc.vector.tensor_scalar(out=c_col[:], in0=setup_ps[:, 1536:1537],
                                    scalar1=-1.0, scalar2=c0,
                                    op0=MULT, op1=ADD)

            # ---- eval-side setup ----------------------------------------
            ev_nat = pp.tile([128, RT * D], f32)
            nc.sync.dma_start(
                out=ev_nat[:].rearrange("p (t d) -> p t d", d=D),
                in_=x_eval[:].rearrange("(p t) d -> p t d", p=128))
            ev_sq = pp.tile([128, RT * D], f32)
            nc.vector.tensor_mul(ev_sq[:], ev_nat[:], ev_nat[:])
            sq_e = pp.tile([128, RT], f32)
            nc.vector.tensor_reduce(
                out=sq_e[:], in_=ev_sq[:].rearrange("p (t d) -> p t d", d=D),
                axis=X, op=ADD)
            # bias_all[:, rt] = scale*|e|^2 + (c0 - log_bw)
            bias_all = pp.tile([128, RT], f32)
            nc.vector.tensor_scalar(out=bias_all[:], in0=sq_e[:],
                                    scalar1=scale_col[:, 0:1],
                                    scalar2=c_col[:, 0:1],
                                    op0=MULT, op1=ADD)

            # ev_ext[:, rt, :] (bf16): cols 0-15 hi(-2e), 16-31 lo(-2e),
            # 32-47 hi(-2e), 48 = 49 = 1.0, 50-63 zero pad
            m2e = pp.tile([128, RT * D], f32)
            nc.vector.tensor_scalar_mul(m2e[:], ev_nat[:], -2.0)
            ev_ext = pp.tile([128, RT * 64], bf16)
            evv = ev_ext[:].rearrange("p (t s) -> p t s", s=64)
            m2e3 = m2e[:].rearrange("p (t d) -> p t d", d=D)
            nc.vector.memset(evv[:, :, 48:64], 0.0)
            nc.vector.memset(evv[:, :, 48:50], 1.0)
            nc.vector.tensor_copy(evv[:, :, 0:16], m2e3)
            nc.vector.tensor_tensor(out=evv[:, :, 16:32], in0=m2e3,
                                    in1=evv[:, :, 0:16], op=SUB)
            nc.vector.tensor_copy(evv[:, :, 32:48], evv[:, :, 0:16])
            for rt in range(RT):
                nc.tensor.transpose(
                    setup_ps[0:64, rt * 64:(rt + 1) * 64].bitcast(bf16),
                    ev_ext[:, rt * 64:(rt + 1) * 64], idb[:])
            # rows 64-127 duplicate rows 0-63: matmul operands must share a
            # base partition, so the B-half matmuls read evT[64:128]
            evT = pp.tile([128, MS], bf16)
            nc.vector.tensor_copy(evT[0:64, :].bitcast(f32),
                                  setup_ps[0:64, 0:RT * 64])
            nc.vector.tensor_copy(evT[64:128, :], evT[0:64, :])

            # ---- base-side prep (per-chunk slices, bf16) ----------------
            # bs_ext[:, t, :]: cols 0-15 hi(b), 16-31 hi(b), 32-47 lo(b),
            # 48 sqb_hi, 49 sqb_lo, 50-63 zero pad
            bs_nat = pp.tile([128, NBT * D], f32)
            bs_sq = pp.tile([128, NBT * D], f32)
            bs_ext = pp.tile([128, NBT * 64], bf16)
            sq_b = pp.tile([128, NBT], f32)

            def prep_base(ci):
                ts = slice(ci * TPC, (ci + 1) * TPC)
                bsv = bs_ext[:].rearrange("p (t s) -> p t s", s=64)[:, ts]
                nat = bs_nat[:].rearrange("p (t d) -> p t d", d=D)[:, ts]
                sq3 = bs_sq[:].rearrange("p (t d) -> p t d", d=D)[:, ts]
                sqb = sq_b[:].rearrange("p (t o) -> p t o", o=1)[:, ts]
                nc.sync.dma_start(
                    out=nat,
                    in_=x_base[:].rearrange("(p t) d -> p t d", p=128)[:, ts])
                nc.vector.memset(bsv[:, :, 48:64], 0.0)
                nc.vector.tensor_copy(bsv[:, :, 0:16], nat)
                nc.vector.tensor_copy(bsv[:, :, 16:32], bsv[:, :, 0:16])
                nc.vector.tensor_tensor(out=bsv[:, :, 32:48], in0=nat,
                                        in1=bsv[:, :, 0:16], op=SUB)
                nc.vector.tensor_mul(sq3, nat, nat)
                nc.vector.tensor_reduce(out=sqb, in_=sq3, axis=X, op=ADD)
                nc.vector.tensor_copy(bsv[:, :, 48:49], sqb)
                nc.vector.tensor_tensor(out=bsv[:, :, 49:50], in0=sqb,
                                        in1=bsv[:, :, 48:49], op=SUB)

            prep_base(0)

            # ---- main loop ----------------------------------------------
            sums = pp.tile([128, RT * NCH], f32)
            for ci in range(NCH):
                tA = mmp.tile([128, CH], f32, tag="mm")
                tB = mmp.tile([128, CH], f32, tag="mm")
                rhs = rhsp.tile([128, CH // 2], bf16, tag="rhs")
                # stage two 64-section tile groups per bf16 transpose into
                # the first half of tA (bf16 view), then move bit-exact
                for h in range(2):
                    for s in range(SLABS // 2 * h, SLABS // 2 * (h + 1)):
                        nc.tensor.transpose(
                            tA[:, s * 64:(s + 1) * 64].bitcast(bf16),
                            bs_ext[:, (ci * SLABS + s) * 128:
                                   (ci * SLABS + s + 1) * 128],
                            idb[:])
                    nc.vector.tensor_copy(
                        rhs[:, h * CH // 4:(h + 1) * CH // 4].bitcast(f32),
                        tA[:, h * CH // 8:(h + 1) * CH // 8])
                for rt in range(RT):
                    ps = tA if rt % 2 == 0 else tB
                    lhsA = evT[0:64, rt * 128:(rt + 1) * 128]
                    lhsB = evT[64:128, rt * 128:(rt + 1) * 128]
                    half = CH // 4      # 512
                    nc.tensor.matmul(ps[:, 0:half], lhsA,
                                     rhs[0:64, 0:half],
                                     start=True, stop=True)
                    nc.tensor.matmul(ps[:, half:2 * half], lhsA,
                                     rhs[0:64, half:2 * half],
                                     start=True, stop=True)
                    nc.tensor.matmul(ps[:, 2 * half:3 * half], lhsB,
                                     rhs[64:128, 0:half],
                                     start=True, stop=True)
                    nc.tensor.matmul(ps[:, 3 * half:4 * half], lhsB,
                                     rhs[64:128, half:2 * half],
                                     start=True, stop=True)
                    nc.scalar.activation(
                        ps[:, :], ps[:, :], Exp,
                        bias=bias_all[:, rt:rt + 1],
                        scale=scale_col[:, 0:1],
                        accum_out=sums[:, rt * NCH + ci:rt * NCH + ci + 1])
                    if rt == 0 and ci + 1 < NCH:
                        prep_base(ci + 1)

            # ---- finalize -----------------------------------------------
            val = pp.tile([128, RT], f32)
            nc.vector.tensor_reduce(
                out=val[:],
                in_=sums[:].rearrange("p (r c) -> p r c", c=NCH),
                axis=X, op=ADD)
            nc.sync.dma_start(out=out[:], in_=val[:])

    nc.compile()
    return nc


def kernel(x_eval, x_base, log_bw):
    from concourse.bass_utils import run_bass_kernel_spmd

    if "nc" not in _CACHE:
        _CACHE["nc"] = _build_nc()
    nc = _CACHE["nc"]

    x_eval = np.ascontiguousarray(x_eval, dtype=np.float32)
    x_base = np.ascontiguousarray(x_base, dtype=np.float32)
    lb = np.asarray(log_bw, dtype=np.float32).reshape(1, 1)
    in_maps = [
        {
            "x_eval": x_eval[i * MS:(i + 1) * MS],
            "x_base": x_base,
            "log_bw": lb,
        }
        for i in range(NCORES)
    ]
    res = run_bass_kernel_spmd(nc, in_maps, list(range(NCORES)))
    # out[p, rt] holds eval point p*RT + rt of the shard -> row-major flatten
    shards = [r["out"].reshape(-1) for r in res.results]
    return np.concatenate(shards).astype(np.float32)
